# revision 1
# baseline (speedup 1.0000x reference)
"""Trainium2 Bass kernel, v2: tail-folded layout for full DVE lane use.

Same math as kernel.py. Difference: the y tail rows (y=128..191, 64
rows) of pairs of z-planes are folded into one 128-partition tile —
partitions 0:64 hold the first half of the chunk's planes, partitions
64:128 the second half (with a 2-plane overlap of the input slots so
z-derivative shifts stay uniform in the free dim). Every DVE op then
runs with all 128 lanes. PE matmuls on upper-half K-tiles use weight
copies stored at partition base 64 (legal 32-aligned base).
"""

import sys

sys.path.insert(0, "/opt/trn_rl_repo")

import numpy as np

N = 192
NCORES = 8

MU_REF = 1.8e-5
T_REF = 300.0
PR = 0.72
CP = 1005.0
C1 = N / 2.0
CLN = float(np.log(np.float32(MU_REF) * C1 * C1))
CPR = float(np.float32(CP / PR))
TWO3 = float(np.float32(2.0 / 3.0))


def build_program(nz=24, za=4, zb=4, num_devices=NCORES):
    import concourse.bacc as bacc
    import concourse.mybir as mybir
    from concourse.tile import TileContext

    f32 = mybir.dt.float32
    nt = nz + 2
    nc = bacc.Bacc("TRN2", target_bir_lowering=False, debug=False,
                   num_devices=num_devices)

    u_d = nc.dram_tensor("u", [3, nz + 4, N, N], f32, kind="ExternalInput")
    t_d = nc.dram_tensor("T", [nz + 4, N, N], f32, kind="ExternalInput")
    dyt_d = nc.dram_tensor("dyt", [N, N], f32, kind="ExternalInput")
    out_d = nc.dram_tensor("out", [4, nz, N, N], f32, kind="ExternalOutput")

    with TileContext(nc) as tc:
        with (
            tc.tile_pool(name="wpool", bufs=1) as wpool,
            tc.tile_pool(name="dram", bufs=1, space="DRAM") as dpool,
            tc.tile_pool(name="psum", bufs=4, space="PSUM") as pspool,
        ):
            clnt = wpool.tile([128, 1], f32, tag="cln")
            nc.vector.memset(clnt[:, :], CLN)

            # Dy^T blocks; (kt) 0=main K rows y0:128, 1=tail K rows y128:192
            # wd[kt][mt] at base 0; wd64[kt=1][mt] at partition base 64.
            dT = dyt_d.ap()
            wd = {}
            wd64 = {}
            for kt, (k0, nk) in enumerate([(0, 128), (128, 64)]):
                for mt, (m0, nm) in enumerate([(0, 128), (128, 64)]):
                    w = wpool.tile([nk, nm], f32, tag=f"wd{kt}{mt}")
                    nc.sync.dma_start(out=w[:, :],
                                      in_=dT[k0:k0 + nk, m0:m0 + nm])
                    wd[(kt, mt)] = w

            bz = dpool.tile([4, nt, N, N], f32, tag="bz")
            by = dpool.tile([4, nt, N, N], f32, tag="by")
            bx = dpool.tile([4, nt, N, N], f32, tag="bx")

            pe_stg_pool = [None]

            def pe_dy(scr, main_ctr, tail_feed, dy0, dy1, npl):
                """y-derivs of 4 fields x npl planes.

                main_ctr: [128, 4, npl, N]; tail_feed: [64, 4, npl, N]
                (base-0 copy of tail rows). dy0: [128,4,npl,N]; dy1:
                folded [128, 4, npl/2, N] (parts 0:64 first half planes).
                Upper-half tail drains stage through base-0 then DMA-hop.
                """
                h = npl // 2 if npl > 1 else 1
                for p in range(npl):
                    lo = p < h
                    for f0 in (0, 2):
                        nw = 2 * N
                        ps = pspool.tile([128, nw], f32, tag="ps0")
                        nc.tensor.matmul(ps[:, :], wd[(0, 0)][:, :],
                                         main_ctr[:, f0:f0 + 2, p, :],
                                         start=True, stop=False)
                        nc.tensor.matmul(ps[:, :], wd[(1, 0)][:, :],
                                         tail_feed[:, f0:f0 + 2, p, :],
                                         start=False, stop=True)
                        nc.scalar.copy(
                            dy0[:, f0:f0 + 2, p, :],
                            ps[:, :].rearrange("p (f x) -> p f x", f=2))
                        pt = pspool.tile([64, nw], f32, tag="ps1")
                        nc.tensor.matmul(pt[:, :], wd[(0, 1)][:, :],
                                         main_ctr[:, f0:f0 + 2, p, :],
                                         start=True, stop=False)
                        nc.tensor.matmul(pt[:, :], wd[(1, 1)][:, :],
                                         tail_feed[:, f0:f0 + 2, p, :],
                                         start=False, stop=True)
                        ptv = pt[:, :].rearrange("p (f x) -> p f x", f=2)
                        if lo:
                            nc.scalar.copy(dy1[0:64, f0:f0 + 2, p, :], ptv)
                        else:
                            stg = pe_stg_pool[0].tile([64, nw], f32, tag="stg")
                            sgv = stg.rearrange("p (f x) -> p f x", f=2)
                            nc.scalar.copy(sgv[:, :, :], ptv)
                            nc.sync.dma_start(
                                out=dy1[64:128, f0:f0 + 2, p - h, :],
                                in_=sgv[:, :, :])

            def compute_block(mybir, scr, v_ctr, dz, dx, dy, zc, suf):
                """Shared tau/e computation on [128, 4, zc, N] views.
                Returns (rv, ev) with 3-field row blocks / e columns."""
                p = 128
                lt = scr.tile([p, zc * N], f32, tag="lt")
                ltv = lt.rearrange("p (z x) -> p z x", z=zc)
                nc.scalar.activation(ltv[:, :, :], v_ctr[:, 3, :, :],
                                     mybir.ActivationFunctionType.Ln)
                mu = scr.tile([p, zc * N], f32, tag="mu")
                muv = mu.rearrange("p (z x) -> p z x", z=zc)
                nc.scalar.activation(muv[:, :, :], ltv[:, :, :],
                                     mybir.ActivationFunctionType.Exp,
                                     bias=clnt[0:p, :], scale=0.7)
                mut = scr.tile([p, zc * N], f32, tag="mut")
                mutv = mut.rearrange("p (z x) -> p z x", z=zc)
                nc.scalar.mul(mut[:, :], mu[:, :], CPR)

                dv = scr.tile([p, zc * N], f32, tag="dv")
                dvv = dv.rearrange("p (z x) -> p z x", z=zc)
                nc.vector.tensor_add(dvv[:, :, :], dz[:, 0, :, :],
                                     dx[:, 2, :, :])
                dv2 = scr.tile([p, zc * N], f32, tag="lt")
                dvv2 = dv2.rearrange("p (z x) -> p z x", z=zc)
                nc.vector.tensor_add(dvv2[:, :, :], dvv[:, :, :],
                                     dy[:, 1, :, :])
                q = scr.tile([p, zc * N], f32, tag="dv")
                qv = q.rearrange("p (z x) -> p z x", z=zc)
                nc.scalar.mul(q[:, :], dv2[:, :], TWO3)

                egt = scr.tile([p, 3 * zc * N], f32, tag="eg")
                eg = egt.rearrange("p (f z x) -> p f z x", f=3, z=zc)
                nc.vector.tensor_mul(eg[:, 0, :, :], mutv[:, :, :],
                                     dz[:, 3, :, :])
                nc.vector.tensor_mul(eg[:, 1, :, :], mutv[:, :, :],
                                     dy[:, 3, :, :])
                nc.vector.tensor_mul(eg[:, 2, :, :], mutv[:, :, :],
                                     dx[:, 3, :, :])

                rv = []
                for i in range(3):
                    rt = scr.tile([p, 3 * zc * N], f32, tag=f"r{i}")
                    rv.append(rt.rearrange("p (f z x) -> p f z x",
                                           f=3, z=zc))
                hb = scr.tile([p, 3 * zc * N], f32, tag="hb")
                hv = hb.rearrange("p (f z x) -> p f z x", f=3, z=zc)
                stt = nc.vector.scalar_tensor_tensor
                mub3 = muv.unsqueeze(1).broadcast_to((p, 3, zc, N))
                mub2 = muv.unsqueeze(1).broadcast_to((p, 2, zc, N))
                stt(hv[:, 0, :, :], dz[:, 0, :, :], 2.0, qv[:, :, :],
                    mybir.AluOpType.mult, mybir.AluOpType.subtract)
                nc.vector.tensor_add(hv[:, 1, :, :], dy[:, 0, :, :],
                                     dz[:, 1, :, :])
                nc.vector.tensor_add(hv[:, 2, :, :], dx[:, 0, :, :],
                                     dz[:, 2, :, :])
                nc.vector.tensor_mul(rv[0][:, :, :, :], hv[:, :, :, :], mub3)
                stt(hv[:, 1, :, :], dy[:, 1, :, :], 2.0, qv[:, :, :],
                    mybir.AluOpType.mult, mybir.AluOpType.subtract)
                nc.vector.tensor_add(hv[:, 2, :, :], dx[:, 1, :, :],
                                     dy[:, 2, :, :])
                nc.vector.tensor_mul(rv[1][:, 1:3, :, :],
                                     hv[:, 1:3, :, :], mub2)
                nc.sync.dma_start(out=rv[1][:, 0, :, :],
                                  in_=rv[0][:, 1, :, :])
                stt(hv[:, 2, :, :], dx[:, 2, :, :], 2.0, qv[:, :, :],
                    mybir.AluOpType.mult, mybir.AluOpType.subtract)
                nc.vector.tensor_mul(rv[2][:, 2, :, :], hv[:, 2, :, :],
                                     muv[:, :, :])
                nc.sync.dma_start(out=rv[2][:, 0, :, :],
                                  in_=rv[0][:, 2, :, :])
                nc.sync.dma_start(out=rv[2][:, 1, :, :],
                                  in_=rv[1][:, 2, :, :])

                pb = scr.tile([p, 3 * zc * N], f32, tag="dx")
                pbv = pb.rearrange("p (f z x) -> p f z x", f=3, z=zc)
                accs = [eg]
                for i in range(3):
                    ui = v_ctr[:, i:i + 1, :, :].broadcast_to((p, 3, zc, N))
                    nc.vector.tensor_mul(pbv[:, :, :, :],
                                         rv[i][:, :, :, :], ui)
                    na = scr.tile([p, 3 * zc * N], f32,
                                  tag=("dz" if i % 2 == 0 else "hb"))
                    nav = na.rearrange("p (f z x) -> p f z x", f=3, z=zc)
                    nc.vector.tensor_add(nav[:, :, :, :],
                                         accs[-1][:, :, :, :],
                                         pbv[:, :, :, :])
                    accs.append(nav)
                return rv, accs[-1]

            import concourse.mybir as mybir_mod

            # =============== PASS A ===============
            pass_a = tc.tile_pool(name="a_io", bufs=2)
            iopool = pass_a.__enter__()
            pe_stg_pool[0] = iopool
            scr_cm = tc.tile_pool(name="a_scr", bufs=1)
            scr = scr_cm.__enter__()
            t = -1
            while t < nz + 1:
                cza = min(za, nz + 1 - t)
                assert cza % 2 == 0, "za and nt must keep chunks even"
                hc = cza // 2
                ip0 = t + 1

                # main input [128, 4, cza+2, N]
                ti0 = iopool.tile([128, 4 * (cza + 2) * N], f32, tag="in0")
                v0 = ti0.rearrange("p (f z x) -> p f z x", f=4, z=cza + 2)
                for fi in range(3):
                    nc.sync.dma_start(
                        out=v0[:, fi, :, :],
                        in_=u_d.ap()[fi, ip0:ip0 + cza + 2, 0:128, :]
                        .transpose([1, 0, 2]))
                nc.sync.dma_start(
                    out=v0[:, 3, :, :],
                    in_=t_d.ap()[ip0:ip0 + cza + 2, 0:128, :]
                    .transpose([1, 0, 2]))
                # folded tail input [128, 4, hc+2, N]
                ti1 = iopool.tile([128, 4 * (hc + 2) * N], f32, tag="in1")
                v1 = ti1.rearrange("p (f z x) -> p f z x", f=4, z=hc + 2)
                for half, pofs in ((0, 0), (1, 64)):
                    p0 = ip0 + half * hc
                    for fi in range(3):
                        nc.sync.dma_start(
                            out=v1[pofs:pofs + 64, fi, :, :],
                            in_=u_d.ap()[fi, p0:p0 + hc + 2, 128:192, :]
                            .transpose([1, 0, 2]))
                    nc.sync.dma_start(
                        out=v1[pofs:pofs + 64, 3, :, :],
                        in_=t_d.ap()[p0:p0 + hc + 2, 128:192, :]
                        .transpose([1, 0, 2]))

                # base-0 tail feed for PE (duplicate load of center rows)
                tft = iopool.tile([64, 4 * cza * N], f32, tag="tf")
                tf = tft.rearrange("p (f z x) -> p f z x", f=4, z=cza)
                for fi in range(3):
                    nc.sync.dma_start(
                        out=tf[:, fi, :, :],
                        in_=u_d.ap()[fi, ip0 + 1:ip0 + 1 + cza, 128:192, :]
                        .transpose([1, 0, 2]))
                nc.sync.dma_start(
                    out=tf[:, 3, :, :],
                    in_=t_d.ap()[ip0 + 1:ip0 + 1 + cza, 128:192, :]
                    .transpose([1, 0, 2]))

                # PE y-derivs
                d0t = iopool.tile([128, 4 * cza * N], f32, tag="dy0")
                dy0 = d0t.rearrange("p (f z x) -> p f z x", f=4, z=cza)
                d1t = iopool.tile([128, 4 * hc * N], f32, tag="dy1")
                dy1 = d1t.rearrange("p (f z x) -> p f z x", f=4, z=hc)
                pe_dy(scr, v0[:, :, 1:1 + cza, :], tf, dy0, dy1, cza)

                for (vv, dyv, zc, suf) in ((v0, dy0, cza, "A"),
                                           (v1, dy1, hc, "B")):
                    ctr = vv[:, :, 1:1 + zc, :]
                    dzt = scr.tile([128, 4 * zc * N], f32, tag="dz")
                    dz = dzt.rearrange("p (f z x) -> p f z x", f=4, z=zc)
                    nc.vector.tensor_sub(dz[:, :, :, :],
                                         vv[:, :, 2:2 + zc, :],
                                         vv[:, :, 0:zc, :])
                    dxt = scr.tile([128, 4 * zc * N], f32, tag="dx")
                    dx = dxt.rearrange("p (f z x) -> p f z x", f=4, z=zc)
                    nc.vector.tensor_sub(dx[:, :, :, 1:191],
                                         ctr[:, :, :, 2:192],
                                         ctr[:, :, :, 0:190])
                    nc.vector.tensor_sub(dx[:, :, :, 0:192:191],
                                         ctr[:, :, :, 1::-1],
                                         ctr[:, :, :, 191:189:-1])

                    rv, ev = compute_block(mybir_mod, scr, ctr, dz, dx,
                                           dyv, zc, suf)

                    tt0 = t + 1
                    for buf, row in ((bz, 0), (by, 1), (bx, 2)):
                        if suf == "A":
                            for fi in range(3):
                                nc.sync.dma_start(
                                    out=buf[fi, tt0:tt0 + zc, 0:128, :]
                                    .transpose([1, 0, 2]),
                                    in_=rv[row][:, fi, :, :])
                            nc.sync.dma_start(
                                out=buf[3, tt0:tt0 + zc, 0:128, :]
                                .transpose([1, 0, 2]),
                                in_=ev[:, row, :, :])
                        else:
                            for half, pofs in ((0, 0), (1, 64)):
                                s0 = tt0 + half * hc
                                for fi in range(3):
                                    nc.sync.dma_start(
                                        out=buf[fi, s0:s0 + hc, 128:192, :]
                                        .transpose([1, 0, 2]),
                                        in_=rv[row][pofs:pofs + 64, fi, :, :])
                                nc.sync.dma_start(
                                    out=buf[3, s0:s0 + hc, 128:192, :]
                                    .transpose([1, 0, 2]),
                                    in_=ev[pofs:pofs + 64, row, :, :])
                t += cza

            scr_cm.__exit__(None, None, None)
            pass_a.__exit__(None, None, None)

            # =============== PASS B ===============
            pass_b = tc.tile_pool(name="b_io", bufs=2)
            iopool = pass_b.__enter__()
            pe_stg_pool[0] = iopool
            scrb_cm = tc.tile_pool(name="b_scr", bufs=1)
            scr = scrb_cm.__enter__()
            z = 0
            while z < nz:
                czb = min(zb, nz - z)
                assert czb % 2 == 0
                hb = czb // 2
                tt0 = z + 1

                lz0t = iopool.tile([128, 4 * (czb + 2) * N], f32, tag="lz0")
                lz0 = lz0t.rearrange("p (f z x) -> p f z x", f=4, z=czb + 2)
                for fi in range(4):
                    nc.sync.dma_start(
                        out=lz0[:, fi, :, :],
                        in_=bz[fi, tt0 - 1:tt0 + czb + 1, 0:128, :]
                        .transpose([1, 0, 2]))
                lz1t = iopool.tile([128, 4 * (hb + 2) * N], f32, tag="lz1")
                lz1 = lz1t.rearrange("p (f z x) -> p f z x", f=4, z=hb + 2)
                for half, pofs in ((0, 0), (1, 64)):
                    s0 = tt0 - 1 + half * hb
                    for fi in range(4):
                        nc.sync.dma_start(
                            out=lz1[pofs:pofs + 64, fi, :, :],
                            in_=bz[fi, s0:s0 + hb + 2, 128:192, :]
                            .transpose([1, 0, 2]))

                ly0t = iopool.tile([128, 4 * czb * N], f32, tag="ly0")
                ly0 = ly0t.rearrange("p (f z x) -> p f z x", f=4, z=czb)
                lx0t = iopool.tile([128, 4 * czb * N], f32, tag="lx0")
                lx0 = lx0t.rearrange("p (f z x) -> p f z x", f=4, z=czb)
                for buf, dst in ((by, ly0), (bx, lx0)):
                    for fi in range(4):
                        nc.sync.dma_start(
                            out=dst[:, fi, :, :],
                            in_=buf[fi, tt0:tt0 + czb, 0:128, :]
                            .transpose([1, 0, 2]))
                ly1t = scr.tile([64, 4 * czb * N], f32, tag="ly1")
                ly1 = ly1t.rearrange("p (f z x) -> p f z x", f=4, z=czb)
                for fi in range(4):
                    nc.sync.dma_start(
                        out=ly1[:, fi, :, :],
                        in_=by[fi, tt0:tt0 + czb, 128:192, :]
                        .transpose([1, 0, 2]))
                lx1t = scr.tile([128, 4 * hb * N], f32, tag="lx1")
                lx1 = lx1t.rearrange("p (f z x) -> p f z x", f=4, z=hb)
                for half, pofs in ((0, 0), (1, 64)):
                    s0 = tt0 + half * hb
                    for fi in range(4):
                        nc.sync.dma_start(
                            out=lx1[pofs:pofs + 64, fi, :, :],
                            in_=bx[fi, s0:s0 + hb, 128:192, :]
                            .transpose([1, 0, 2]))

                d0t = iopool.tile([128, 4 * czb * N], f32, tag="db0")
                dy0 = d0t.rearrange("p (f z x) -> p f z x", f=4, z=czb)
                d1t = iopool.tile([128, 4 * hb * N], f32, tag="db1")
                dy1 = d1t.rearrange("p (f z x) -> p f z x", f=4, z=hb)
                pe_dy(scr, ly0, ly1, dy0, dy1, czb)

                for (lzv, lxv, dyv, zc, half_mode) in (
                        (lz0, lx0, dy0, czb, False),
                        (lz1, lx1, dy1, hb, True)):
                    suf = "B" if half_mode else "A"
                    mt_ = scr.tile([128, 4 * zc * N], f32, tag="mb")
                    mv = mt_.rearrange("p (f z x) -> p f z x", f=4, z=zc)
                    nc.vector.tensor_sub(mv[:, :, :, :],
                                         lzv[:, :, 2:2 + zc, :],
                                         lzv[:, :, 0:zc, :])
                    xt_ = scr.tile([128, 4 * zc * N], f32, tag="xb")
                    xv = xt_.rearrange("p (f z x) -> p f z x", f=4, z=zc)
                    nc.vector.tensor_sub(xv[:, :, :, 1:191],
                                         lxv[:, :, :, 2:192],
                                         lxv[:, :, :, 0:190])
                    nc.vector.tensor_sub(xv[:, :, :, 0:192:191],
                                         lxv[:, :, :, 1::-1],
                                         lxv[:, :, :, 191:189:-1])
                    st_ = scr.tile([128, 4 * zc * N], f32, tag="ly1")
                    sv = st_.rearrange("p (f z x) -> p f z x", f=4, z=zc)
                    nc.vector.tensor_add(sv[:, :, :, :], mv[:, :, :, :],
                                         xv[:, :, :, :])
                    ot = scr.tile([128, 4 * zc * N], f32, tag="xb2")
                    ov = ot.rearrange("p (f z x) -> p f z x", f=4, z=zc)
                    nc.vector.tensor_add(ov[:, :, :, :], sv[:, :, :, :],
                                         dyv[:, :, :, :])
                    if not half_mode:
                        for fi in range(4):
                            nc.sync.dma_start(
                                out=out_d.ap()[fi, z:z + zc, 0:128, :]
                                .transpose([1, 0, 2]),
                                in_=ov[:, fi, :, :])
                    else:
                        for half, pofs in ((0, 0), (1, 64)):
                            s0 = z + half * hb
                            for fi in range(4):
                                nc.sync.dma_start(
                                    out=out_d.ap()[fi, s0:s0 + hb,
                                                   128:192, :]
                                    .transpose([1, 0, 2]),
                                    in_=ov[pofs:pofs + 64, fi, :, :])
                z += czb

            scrb_cm.__exit__(None, None, None)
            pass_b.__exit__(None, None, None)

    nc.compile()
    return nc


_NC_CACHE = None


def _get_nc():
    global _NC_CACHE
    if _NC_CACHE is None:
        _NC_CACHE = build_program()
    return _NC_CACHE


def make_dyt() -> np.ndarray:
    dm = np.zeros((N, N), dtype=np.float32)
    for m in range(N):
        dm[m, (m + 1) % N] = 1.0
        dm[m, (m - 1) % N] = -1.0
    return np.ascontiguousarray(dm.T)


def shard_inputs(u, T, nz=24, ncores=NCORES):
    dyt = make_dyt()
    in_maps = []
    for k in range(ncores):
        idx = np.arange(nz * k - 2, nz * k + nz + 2) % N
        in_maps.append({
            "u": np.ascontiguousarray(u[:, idx, :, :]),
            "T": np.ascontiguousarray(T[idx, :, :]),
            "dyt": dyt,
        })
    return in_maps


def kernel(u: np.ndarray, T: np.ndarray) -> np.ndarray:
    from concourse.bass_utils import run_bass_kernel_spmd

    u = np.asarray(u, dtype=np.float32)
    T = np.asarray(T, dtype=np.float32)
    nc = _get_nc()
    nz = N // NCORES
    in_maps = shard_inputs(u, T, nz=nz)
    res = run_bass_kernel_spmd(nc, in_maps, list(range(NCORES)))

    out = np.zeros((5, N, N, N), dtype=np.float32)
    for k in range(NCORES):
        out[1:5, nz * k:nz * k + nz, :, :] = res.results[k]["out"]
    return out



# revision 3
# speedup vs baseline: 4.9380x; 4.9380x over previous
"""Trainium2 Bass kernel v4: fused single-pass compressible-NS RHS.

Like v3 (fused chunks, bf16, folded tail, PE divergence) plus:
- ln/exp hoisted per chunk (2 activation-table loads per chunk, not 4)
- input z-derivative on PE (+/-identity matmuls) drained by ACT
- e-chain multiplies batched into 3-field ops; e_x mul on GpSimd
- slimmed carry copies / tail hop DMAs
- all weights packed into one DMA
"""

import sys

sys.path.insert(0, "/opt/trn_rl_repo")

import numpy as np

N = 192
NCORES = 8
CZ = 4  # flux planes computed per chunk

MU_REF = 1.8e-5
T_REF = 300.0
PR = 0.72
CP = 1005.0
C1 = N / 2.0
CPR = float(np.float32(CP / PR))
# mu' = K*sqrt(q(T)), q(T) = QA2*T^2 + QB*T + QG ~ T^1.4 on [0.5, 1.5]
QA = float(np.sqrt(0.30153127))
QB = 0.7893985
QG = -0.09243873
K2 = float((np.float32(MU_REF) * C1 * C1) ** 2)

# packed weight layout: columns [wd00 128][wd10x2 128][wbd 128][wi 128]
#                               [wn 128][wd01 64]
WCOLS = 128 * 5 + 64


def build_program(nz=24, num_devices=NCORES):
    import concourse.bacc as bacc
    import concourse.mybir as mybir
    from concourse.tile import TileContext

    assert nz % CZ == 0
    nch = nz // CZ

    bf = mybir.dt.bfloat16
    f32 = mybir.dt.float32
    Act = mybir.ActivationFunctionType
    nc = bacc.Bacc("TRN2", target_bir_lowering=False, debug=False,
                   num_devices=num_devices)

    in_d = nc.dram_tensor("in0", [nz + 4, N, 4, N], bf, kind="ExternalInput")
    wts_d = nc.dram_tensor("wts", [128, WCOLS], bf, kind="ExternalInput")
    out_d = nc.dram_tensor("out", [nz, N, 4, N], bf, kind="ExternalOutput")

    inv = in_d.ap()
    outv = out_d.ap()

    with TileContext(nc) as tc:
        with (
            tc.tile_pool(name="wpool", bufs=1) as wpool,
            tc.tile_pool(name="per", bufs=1) as per,
            tc.tile_pool(name="iop", bufs=2) as iop,
            tc.tile_pool(name="scr", bufs=1) as scr,
            tc.tile_pool(name="pin", bufs=2, space="PSUM") as pin,
            tc.tile_pool(name="pem", bufs=2, space="PSUM") as pem,
        ):
            wt = wpool.tile([128, WCOLS], bf, tag="wts")
            nc.sync.dma_start(out=wt[:, :], in_=wts_d.ap()[:, :])
            wd00 = wt[:, 0:128]
            wd10 = wt[:, 128:256]      # stacked base-0 / base-64 copies
            wbd = wt[:, 256:384]
            wi = wt[:, 384:512]
            wn = wt[:, 512:640]
            wd01 = wt[:, 640:704]

            # persistent flux tiles
            FM_t = per.tile([128, 3 * 4 * 6 * N], bf, tag="FM")
            FM = FM_t.rearrange("p (d f s x) -> p d f s x", d=3, f=4, s=6)
            TE_t = per.tile([128, 3 * 4 * 4 * N], bf, tag="TE")
            TE = TE_t.rearrange("p (d f s x) -> p d f s x", d=3, f=4, s=4)
            FMg = FM_t.rearrange("p (g s x) -> p g s x", g=12, s=6)
            TEg = TE_t.rearrange("p (g s x) -> p g s x", g=12, s=4)

            # ---------- helpers ----------
            def mk(pool, tag, dims):
                free = 1
                for d in dims:
                    free *= d
                t = pool.tile([128, free], bf, tag=tag)
                pat = " ".join(chr(97 + i) for i in range(len(dims)))
                return t.rearrange(
                    f"p ({pat}) -> p {pat}",
                    **{chr(97 + i): dims[i] for i in range(len(dims))})

            def indz_main(v0, GD, zslots, v1, v1slots, v1half):
                """in-plane y-deriv + z-deriv for main rows; one fused
                drain per (plane, fpair) into GD[:, i, (g1|dz), f, x]."""
                for i, zp in enumerate(zslots):
                    h = v1half[i]
                    w10 = wd10[0:64, 0:128] if h == 0 else wd10[64:128, 0:128]
                    v1s = v1[0:64] if h == 0 else v1[64:128]
                    for fp in range(2):
                        ps = pin.tile([128, 1024], f32, tag="pind")
                        psv = ps.rearrange("p (b k) -> p b k", b=2)
                        fsl = slice(2 * fp, 2 * fp + 2)
                        ra = psv[:, 0, 0:384].rearrange("p (f x) -> p f x",
                                                        f=2)
                        rb = psv[:, 1, 0:384].rearrange("p (f x) -> p f x",
                                                        f=2)
                        nc.tensor.matmul(ra[:, :, :], wd00,
                                         v0[:, zp, fsl, :],
                                         start=True, stop=False,
                                         skip_group_check=True)
                        nc.tensor.matmul(ra[:, :, :], w10,
                                         v1s[:, v1slots[i], fsl, :],
                                         start=False, stop=True,
                                         skip_group_check=True)
                        nc.tensor.matmul(rb[:, :, :], wi,
                                         v0[:, zp + 1, fsl, :],
                                         start=True, stop=False,
                                         skip_group_check=True)
                        nc.tensor.matmul(rb[:, :, :], wn,
                                         v0[:, zp - 1, fsl, :],
                                         start=False, stop=True,
                                         skip_group_check=True)
                        dst = GD[:, i, :, fsl, :]
                        srcv = psv[:, :, 0:384].rearrange(
                            "p b (f x) -> p b f x", f=2)
                        nc.scalar.copy(dst[:, :, :, :], srcv[:, :, :, :])

            def indz_tail(v0, GD, pair_specs):
                for (gs, z0, z1, v1t, v1s) in pair_specs:
                    for fp in range(2):
                        ps = pin.tile([128, 1024], f32, tag="pind")
                        psv = ps.rearrange("p (b k) -> p b k", b=2)
                        fsl = slice(2 * fp, 2 * fp + 2)
                        ra = psv[:, 0, 0:384].rearrange("p (f x) -> p f x",
                                                        f=2)
                        rb = psv[:, 1, 0:384].rearrange("p (f x) -> p f x",
                                                        f=2)
                        nc.tensor.matmul(ra[0:64, :, :], wd01,
                                         v0[:, z0, fsl, :],
                                         start=True, stop=True,
                                         skip_group_check=True)
                        nc.tensor.matmul(ra[64:128, :, :], wd01,
                                         v0[:, z1, fsl, :],
                                         start=True, stop=True,
                                         skip_group_check=True)
                        nc.tensor.matmul(ra[:, :, :], wbd,
                                         v1t[:, v1s, fsl, :],
                                         start=False, stop=True,
                                         skip_group_check=True)
                        nc.tensor.matmul(rb[:, :, :], wi,
                                         v1t[:, v1s + 1, fsl, :],
                                         start=True, stop=False,
                                         skip_group_check=True)
                        nc.tensor.matmul(rb[:, :, :], wn,
                                         v1t[:, v1s - 1, fsl, :],
                                         start=False, stop=True,
                                         skip_group_check=True)
                        dst = GD[:, gs, :, fsl, :]
                        srcv = psv[:, :, 0:384].rearrange(
                            "p b (f x) -> p b f x", f=2)
                        nc.scalar.copy(dst[:, :, :, :], srcv[:, :, :, :])

            def mu_block(muM, muT, TM, TT_, sM, sT):
                """mu (and CPR*mu) for main+tail via Square/Sqrt only."""
                nc.scalar.activation(sM, TM, Act.Square, scale=QA)
                nc.scalar.activation(sT, TT_, Act.Square, scale=QA)
                for (mv, s_, Tv) in ((muM, sM, TM), (muT, sT, TT_)):
                    nc.vector.tensor_scalar(mv[:, 1], Tv, QB, QG,
                                            mybir.AluOpType.mult,
                                            mybir.AluOpType.add)
                    nc.vector.tensor_add(mv[:, 1], mv[:, 1], s_)
                    nc.scalar.activation(mv[:, 0], mv[:, 1], Act.Sqrt,
                                         scale=K2)
                    nc.vector.tensor_scalar_mul(mv[:, 1], mv[:, 0], CPR)
                    nc.vector.tensor_scalar_mul(mv[:, 2], mv[:, 0], 2.0)

            def tau_stage(v, g1, dz, zc, mus, Fz, Fy, Fx, F3, Fz12v,
                          ub, psl=None):
                """Flux for zc planes. mus: [P, 2, zc, N] (mu, mut).
                F3: d -> 3-field flux block; Fy2: Fy fields 1:3 block."""
                ctr = v[:, 1:1 + zc, :, :]
                mu, mut = mus[:, 0], mus[:, 1]

                dxv = mk(scr, "dx", (4, 4, N))
                dxv = dxv[psl] if psl else dxv
                dx = dxv[:, 0:zc]
                nc.vector.tensor_sub(dx[:, :, :, 1:191],
                                     ctr[:, :, :, 2:192], ctr[:, :, :, 0:190])
                nc.vector.tensor_sub(dx[:, :, :, 0:192:191],
                                     ctr[:, :, :, 1::-1],
                                     ctr[:, :, :, 191:189:-1])

                qv = mk(scr, "q", (2, 4, N))
                qv = qv[psl] if psl else qv
                qv = qv[:, :, 0:zc]
                dv, qh = qv[:, 0], qv[:, 1]
                nc.gpsimd.tensor_add(dv[:, :, :], dz[:, :, 0, :],
                                     g1[:, :, 1, :])
                nc.gpsimd.tensor_add(dv[:, :, :], dv[:, :, :],
                                     dx[:, :, 2, :])
                nc.vector.tensor_scalar_mul(qh[:, :, :], dv[:, :, :],
                                            float(1.0 / 3.0))

                # off-diagonal sums (DVE) and g-qh diagonals (Pool),
                # written pre-mu straight into the flux slots
                mu2 = mus[:, 2]
                nc.vector.tensor_add(Fz[1], g1[:, :, 0, :], dz[:, :, 1, :])
                nc.vector.tensor_add(Fz[2], dx[:, :, 0, :], dz[:, :, 2, :])
                nc.vector.tensor_add(Fy[2], dx[:, :, 1, :], g1[:, :, 2, :])
                nc.gpsimd.tensor_sub(Fz[0], dz[:, :, 0, :], qh[:, :, :])
                nc.gpsimd.tensor_sub(Fy[1], g1[:, :, 1, :], qh[:, :, :])
                nc.gpsimd.tensor_sub(Fx[2], dx[:, :, 2, :], qh[:, :, :])

                # scale by mu (mu2 on diagonals) in place
                P = 64 if psl else 128
                mu_b2 = mus[:, 0:1].broadcast_to((P, 2, zc, N))
                nc.vector.tensor_mul(Fz[0], Fz[0], mu2)
                nc.vector.tensor_mul(Fz12v, Fz12v, mu_b2)
                nc.vector.tensor_mul(Fy[1], Fy[1], mu2)
                nc.vector.tensor_mul(Fy[2], Fy[2], mu)
                nc.vector.tensor_mul(Fx[2], Fx[2], mu2)
                nc.vector.tensor_copy(Fy[0], Fz[1])
                nc.vector.tensor_copy(Fx[0], Fz[2])
                nc.vector.tensor_copy(Fx[1], Fy[2])

                # energy fluxes: e_j = sum_i tau_ij u_i + mut * DjT
                pr = mk(scr, "pr", (3, 4, N))
                pr = pr[psl] if psl else pr
                pr = pr[:, :, 0:zc]
                t0v = mk(scr, "t0", (2, 4, N))
                t0v = t0v[psl] if psl else t0v
                t0v = t0v[:, :, 0:zc]
                t0, t1 = t0v[:, 0], t0v[:, 1]

                # e_z on DVE
                nc.vector.tensor_mul(pr[:, :, :, :], F3[0], ub)
                nc.vector.tensor_add(t0, pr[:, 0], pr[:, 1])
                nc.vector.tensor_mul(t1, mut, dz[:, :, 3, :])
                nc.vector.tensor_add(t0, t0, pr[:, 2])
                nc.vector.tensor_add(Fz[3], t0, t1)
                # e_x: batched mul on GpSimd, rest on DVE
                prx = mk(scr, "prx", (3, 4, N))
                prx = prx[psl] if psl else prx
                prx = prx[:, :, 0:zc]
                nc.gpsimd.tensor_mul(prx[:, :, :, :], F3[2], ub)
                nc.vector.tensor_add(t0, prx[:, 0], prx[:, 1])
                nc.vector.tensor_mul(t1, mut, dx[:, :, 3, :])
                nc.vector.tensor_add(t0, t0, prx[:, 2])
                nc.vector.tensor_add(Fx[3], t0, t1)
                # e_y fully on GpSimd
                pry = mk(scr, "pry", (3, 4, N))
                pry = pry[psl] if psl else pry
                pry = pry[:, :, 0:zc]
                g = nc.gpsimd
                g.tensor_mul(pry[:, :, :, :], F3[1], ub)
                g.tensor_add(pry[:, 0], pry[:, 0], pry[:, 1])
                g.tensor_mul(pry[:, 1], mut, g1[:, :, 3, :])
                g.tensor_add(pry[:, 0], pry[:, 0], pry[:, 2])
                g.tensor_add(Fy[3], pry[:, 0], pry[:, 1])

            def indy_main(v0, g1M, zslots, v1, v1slots, v1half):
                for i, zp in enumerate(zslots):
                    h = v1half[i]
                    w10 = wd10[0:64, 0:128] if h == 0 else wd10[64:128, 0:128]
                    v1s = v1[0:64] if h == 0 else v1[64:128]
                    for fp in range(2):
                        ps = pin.tile([128, 512], f32, tag="pin")
                        reg = ps[:, 0:384].rearrange("p (f x) -> p f x", f=2)
                        nc.tensor.matmul(reg[:, :, :], wd00,
                                         v0[:, zp, 2 * fp:2 * fp + 2, :],
                                         start=True, stop=False)
                        nc.tensor.matmul(reg[:, :, :], w10,
                                         v1s[:, v1slots[i],
                                             2 * fp:2 * fp + 2, :],
                                         start=False, stop=True)
                        nc.scalar.copy(g1M[:, i, 2 * fp:2 * fp + 2, :],
                                       reg[:, :, :])

            def indy_tail(v0, g1T, pair_specs):
                for (gs, z0, z1, v1t, v1s) in pair_specs:
                    for fp in range(2):
                        ps = pin.tile([128, 512], f32, tag="pin")
                        reg = ps[:, 0:384].rearrange("p (f x) -> p f x", f=2)
                        nc.tensor.matmul(reg[0:64, :, :], wd01,
                                         v0[:, z0, 2 * fp:2 * fp + 2, :],
                                         start=True, stop=True)
                        nc.tensor.matmul(reg[64:128, :, :], wd01,
                                         v0[:, z1, 2 * fp:2 * fp + 2, :],
                                         start=True, stop=True,
                                         skip_group_check=True)
                        nc.tensor.matmul(reg[:, :, :], wbd,
                                         v1t[:, v1s, 2 * fp:2 * fp + 2, :],
                                         start=False, stop=True,
                                         skip_group_check=True)
                        nc.scalar.copy(g1T[:, gs, 2 * fp:2 * fp + 2, :],
                                       reg[:, :, :])

            # ================= PROLOGUE =================
            v0p = mk(iop, "v0", (6, 4, N))
            nc.sync.dma_start(
                out=v0p[:, 0:4].rearrange("p z f x -> p z (f x)"),
                in_=inv[0:4, 0:128, :, :]
                .rearrange("z p f x -> z p (f x)").transpose([1, 0, 2]))
            v1p = mk(iop, "v1", (4, 4, N))
            nc.sync.dma_start(
                out=v1p[64:128].rearrange("p z f x -> p z (f x)"),
                in_=inv[0:4, 128:192, :, :]
                .rearrange("z p f x -> z p (f x)").transpose([1, 0, 2]))

            g1P = mk(scr, "g1M", (CZ, 4, N))
            indy_main(v0p, g1P, [1, 2], v1p, [1, 2], [1, 1])
            g1PT = mk(scr, "g1T", (2, 4, N))
            for i, zp in enumerate([1, 2]):
                for fp in range(2):
                    ps = pin.tile([128, 512], f32, tag="pin")
                    reg = ps[:, 0:384].rearrange("p (f x) -> p f x", f=2)
                    nc.tensor.matmul(reg[64:128, :, :], wbd[64:128, 64:128],
                                     v1p[64:128, zp, 2 * fp:2 * fp + 2, :],
                                     start=True, stop=False,
                                     skip_group_check=True)
                    nc.tensor.matmul(reg[64:128, :, :], wd01,
                                     v0p[:, zp, 2 * fp:2 * fp + 2, :],
                                     start=False, stop=True,
                                     skip_group_check=True)
                    nc.scalar.copy(g1PT[64:128, i, 2 * fp:2 * fp + 2, :],
                                   reg[64:128, :, :])

            dzP = mk(scr, "dzM", (CZ, 4, N))
            dz_pe(v0p, dzP, [(1, 0), (2, 1)])
            dzPT = mk(scr, "dzT", (2, 4, N))
            dz_pe(v1p[64:128], dzPT[64:128], [(1, 0), (2, 1)],
                  psl=slice(64, 128), skip=True)

            muM_t = mk(scr, "muM", (3, CZ, N))
            muT_t = mk(scr, "muT", (3, CZ, N))
            lnM = mk(scr, "lnM", (CZ, N))
            lnT = mk(scr, "lnT", (CZ, N))
            mu_block(muM_t[:, :, 0:2], muT_t[64:128, :, 0:2],
                     v0p[:, 1:3, 3, :], v1p[64:128, 1:3, 3, :],
                     lnM[:, 0:2], lnT[64:128, 0:2],
                     cbT=clnt[64:128])

            psl = slice(64, 128)
            tau_stage(v0p, g1P[:, 0:2], dzP[:, 0:2], 2, muM_t[:, :, 0:2],
                      {i: FM[:, 0, i, 4:6, :] for i in range(4)},
                      {i: FM[:, 1, i, 4:6, :] for i in range(4)},
                      {i: FM[:, 2, i, 4:6, :] for i in range(4)},
                      {d: FM[:, d, 0:3, 4:6, :] for d in range(3)},
                      v0p[:, 1:3, 0:3, :].transpose([0, 2, 1, 3]))
            tau_stage(v1p[64:128], g1PT[64:128], dzPT[64:128], 2,
                      muT_t[64:128, :, 0:2],
                      {i: TE[64:128, 0, i, 2:4, :] for i in range(4)},
                      {i: TE[64:128, 1, i, 2:4, :] for i in range(4)},
                      {i: TE[64:128, 2, i, 2:4, :] for i in range(4)},
                      {d: TE[64:128, d, 0:3, 2:4, :] for d in range(3)},
                      v1p[64:128, 1:3, 0:3, :].transpose([0, 2, 1, 3]),
                      psl=psl)

            # ================= CHUNKS =================
            for k in range(nch):
                a = CZ * k
                i0 = a + 2

                # carry: Fz slots 4,5 -> 0,1 ; Fy/Fx slot 5 -> 1
                nc.vector.tensor_copy(FMg[:, 0:4, 0:2, :],
                                      FMg[:, 0:4, 4:6, :])
                nc.vector.tensor_copy(FMg[:, 4:12, 1:2, :],
                                      FMg[:, 4:12, 5:6, :])
                # hopA: Fz slots 2,3 @hi -> 0,1 @lo ; Fy/Fx slot3 @hi -> 1 @lo
                nc.sync.dma_start(out=TEg[0:64, 0:4, 0:2, :],
                                  in_=TEg[64:128, 0:4, 2:4, :])
                nc.sync.dma_start(out=TEg[0:64, 4:12, 1:2, :],
                                  in_=TEg[64:128, 4:12, 3:4, :])

                v0 = mk(iop, "v0", (6, 4, N))
                nc.sync.dma_start(
                    out=v0.rearrange("p z f x -> p z (f x)"),
                    in_=inv[i0:i0 + 6, 0:128, :, :]
                    .rearrange("z p f x -> z p (f x)").transpose([1, 0, 2]))
                v1 = mk(iop, "v1", (4, 4, N))
                nc.sync.dma_start(
                    out=v1[0:64].rearrange("p z f x -> p z (f x)"),
                    in_=inv[i0:i0 + 4, 128:192, :, :]
                    .rearrange("z p f x -> z p (f x)").transpose([1, 0, 2]))
                nc.sync.dma_start(
                    out=v1[64:128].rearrange("p z f x -> p z (f x)"),
                    in_=inv[i0 + 2:i0 + 6, 128:192, :, :]
                    .rearrange("z p f x -> z p (f x)").transpose([1, 0, 2]))

                g1M = mk(scr, "g1M", (CZ, 4, N))
                indy_main(v0, g1M, [1, 2, 3, 4], v1, [1, 2, 1, 2],
                          [0, 0, 1, 1])
                g1T = mk(scr, "g1T", (2, 4, N))
                indy_tail(v0, g1T, [(0, 1, 3, v1, 1), (1, 2, 4, v1, 2)])

                dzM = mk(scr, "dzM", (CZ, 4, N))
                dz_pe(v0, dzM, [(1, 0), (2, 1), (3, 2), (4, 3)])
                dzT = mk(scr, "dzT", (2, 4, N))
                dz_pe(v1, dzT, [(1, 0), (2, 1)])

                muM_t = mk(scr, "muM", (3, CZ, N))
                muT_t = mk(scr, "muT", (3, CZ, N))
                lnM = mk(scr, "lnM", (CZ, N))
                lnT = mk(scr, "lnT", (CZ, N))
                mu_block(muM_t, muT_t[:, :, 0:2],
                         v0[:, 1:5, 3, :], v1[:, 1:3, 3, :],
                         lnM, lnT[:, 0:2])

                tau_stage(v0, g1M, dzM, CZ, muM_t,
                          {i: FM[:, 0, i, 2:6, :] for i in range(4)},
                          {i: FM[:, 1, i, 2:6, :] for i in range(4)},
                          {i: FM[:, 2, i, 2:6, :] for i in range(4)},
                          {d: FM[:, d, 0:3, 2:6, :] for d in range(3)},
                          v0[:, 1:5, 0:3, :].transpose([0, 2, 1, 3]))
                tau_stage(v1, g1T, dzT, 2, muT_t[:, :, 0:2],
                          {i: TE[:, 0, i, 2:4, :] for i in range(4)},
                          {i: TE[:, 1, i, 2:4, :] for i in range(4)},
                          {i: TE[:, 2, i, 2:4, :] for i in range(4)},
                          {d: TE[:, d, 0:3, 2:4, :] for d in range(3)},
                          v1[:, 1:3, 0:3, :].transpose([0, 2, 1, 3]))

                # hopB: Fz slots 2,3 @lo -> 0,1 @hi ; Fy/Fx slot3 @lo -> 1 @hi
                nc.sync.dma_start(out=TEg[64:128, 0:4, 0:2, :],
                                  in_=TEg[0:64, 0:4, 2:4, :])
                nc.sync.dma_start(out=TEg[64:128, 4:12, 1:2, :],
                                  in_=TEg[0:64, 4:12, 3:4, :])

                # ---- emission main ----
                outM = mk(iop, "outM", (CZ, 4, N))
                for i in range(CZ):
                    s = i + 1
                    if i == 0:
                        teh, tes, w10 = (TEprev[64:128], 1,
                                         wd10[64:128, 0:128])
                    elif i < 3:
                        teh, tes, w10 = TE[0:64], i - 1, wd10[0:64, 0:128]
                    else:
                        teh, tes, w10 = TE[64:128], 0, wd10[64:128, 0:128]
                    ps = pem.tile([128, 1024], f32, tag="pem")
                    psv = ps.rearrange("p (b k) -> p b k", b=2)
                    for fp in range(2):
                        reg = psv[:, fp, 0:384].rearrange(
                            "p (f x) -> p f x", f=2)
                        f0 = 2 * fp
                        nc.tensor.matmul(reg[:, :, :], wi,
                                         FM[:, 0, f0:f0 + 2, s + 1, :],
                                         start=True, stop=False,
                                         skip_group_check=True)
                        nc.tensor.matmul(reg[:, :, :], wd00,
                                         FM[:, 1, f0:f0 + 2, s, :],
                                         start=False, stop=False,
                                         skip_group_check=True)
                        nc.tensor.matmul(reg[:, :, :], w10,
                                         teh[:, 1, f0:f0 + 2, tes, :],
                                         start=False, stop=False,
                                         skip_group_check=True)
                        for ff in range(2):
                            Fxc = FM[:, 2, f0 + ff, s, :]
                            nc.tensor.matmul(reg[:, ff, 0:191], wi,
                                             Fxc[:, 1:192],
                                             start=False, stop=False,
                                         skip_group_check=True)
                            nc.tensor.matmul(reg[:, ff, 191:192], wi,
                                             Fxc[:, 0:1],
                                             start=False, stop=False,
                                         skip_group_check=True)
                            nc.tensor.matmul(reg[:, ff, 1:192], wn,
                                             Fxc[:, 0:191],
                                             start=False, stop=False,
                                         skip_group_check=True)
                            nc.tensor.matmul(reg[:, ff, 0:1], wn,
                                             Fxc[:, 191:192],
                                             start=False, stop=False,
                                         skip_group_check=True)
                        nc.tensor.matmul(reg[:, :, :], wn,
                                         FM[:, 0, f0:f0 + 2, s - 1, :],
                                         start=False, stop=True,
                                         skip_group_check=True)
                    dst = outM[:, i, :, :].rearrange("p (b f) x -> p b f x",
                                                     b=2)
                    srcv = psv[:, :, 0:384].rearrange(
                        "p b (f x) -> p b f x", f=2)
                    nc.scalar.copy(dst[:, :, :, :], srcv[:, :, :, :])

                # ---- emission tail ----
                outT = mk(iop, "outT", (2, 4, N))

                def tloc(prel):
                    """tail flux source for plane a+prel: (tile, half, slot).
                    half 0 = parts 0:64, half 1 = parts 64:128."""
                    if prel <= 0:
                        return (TEprev, 1, prel + 1)
                    if prel <= 2:
                        return (TE, 0, prel - 1)
                    return (TE, 1, prel - 3)

                def wblk(w, sh, oh):
                    del oh
                    return w[64 * sh:64 * sh + 64, 64 * sh:64 * sh + 64]

                for j in range(2):
                    s = j + 1
                    ps = pem.tile([128, 1024], f32, tag="pem")
                    psv = ps.rearrange("p (b k) -> p b k", b=2)
                    for fp in range(2):
                        reg = psv[:, fp, 0:384].rearrange(
                            "p (f x) -> p f x", f=2)
                        f0 = 2 * fp
                        fsl = slice(f0, f0 + 2)
                        started = [False, False]

                        def acc(d, prel_lo, w128, w64, xsl=None, osl=None,
                                last=False):
                            """Accumulate flux dir d at planes
                            (a+prel_lo, a+prel_lo+2) into reg halves."""
                            la, lb = tloc(prel_lo), tloc(prel_lo + 2)
                            fused = (la[0] is lb[0] and la[2] == lb[2]
                                     and la[1] == 0 and lb[1] == 1)
                            xs = xsl if xsl is not None else slice(0, N)
                            os_ = osl if osl is not None else slice(0, N)
                            if fused:
                                mv = la[0][:, d, fsl, la[2], xs]
                                st = not (started[0] and started[1])
                                nc.tensor.matmul(reg[:, :, os_], w128,
                                                 mv, start=st,
                                                 stop=last,
                                                 skip_group_check=True)
                                started[0] = started[1] = True
                                return
                            for oh, (tt, sh, sl) in ((0, la), (1, lb)):
                                mv = tt[64 * sh:64 * sh + 64, d, fsl, sl, xs]
                                r = reg[64 * oh:64 * oh + 64, :, os_]
                                st = not started[oh]
                                nc.tensor.matmul(r, wblk(w64, sh, oh), mv,
                                                 start=st, stop=last,
                                                 skip_group_check=True)
                                started[oh] = True

                        # Fz(p+1), Fz(p-1), Fy(p) via Dy, Fx(p) shifts
                        acc(0, s, wi, wi)
                        acc(0, s - 2, wn, wn)
                        # Dy K0 from main rows
                        nc.tensor.matmul(reg[0:64, :, :], wd01,
                                         FM[:, 1, fsl, s, :],
                                         start=False, stop=False,
                                         skip_group_check=True)
                        nc.tensor.matmul(reg[64:128, :, :], wd01,
                                         FM[:, 1, fsl, s + 2, :],
                                         start=False, stop=False,
                                         skip_group_check=True)
                        # Dy K64 from tail rows
                        acc(1, s - 1, wbd, wbd)
                        # Fx shifts (per field)
                        for ff in range(2):
                            fsl_save = fsl
                            fsl = slice(f0 + ff, f0 + ff + 1)
                            acc(2, s - 1, wi, wi,
                                xsl=slice(1, 192), osl=slice(0, 191))
                            acc(2, s - 1, wi, wi,
                                xsl=slice(0, 1), osl=slice(191, 192))
                            acc(2, s - 1, wn, wn,
                                xsl=slice(0, 191), osl=slice(1, 192))
                            acc(2, s - 1, wn, wn,
                                xsl=slice(191, 192), osl=slice(0, 1),
                                last=(ff == 1))
                            fsl = fsl_save
                    dst = outT[:, j, :, :].rearrange("p (b f) x -> p b f x",
                                                     b=2)
                    srcv = psv[:, :, 0:384].rearrange(
                        "p b (f x) -> p b f x", f=2)
                    nc.scalar.copy(dst[:, :, :, :], srcv[:, :, :, :])

                nc.sync.dma_start(
                    out=outv[a:a + 4, 0:128, :, :]
                    .rearrange("z p f x -> z p (f x)").transpose([1, 0, 2]),
                    in_=outM.rearrange("p z f x -> p z (f x)"))
                nc.sync.dma_start(
                    out=outv[a:a + 2, 128:192, :, :]
                    .rearrange("z p f x -> z p (f x)").transpose([1, 0, 2]),
                    in_=outT[0:64].rearrange("p z f x -> p z (f x)"))
                nc.sync.dma_start(
                    out=outv[a + 2:a + 4, 128:192, :, :]
                    .rearrange("z p f x -> z p (f x)").transpose([1, 0, 2]),
                    in_=outT[64:128].rearrange("p z f x -> p z (f x)"))

    nc.compile()
    return nc


_NC_CACHE = None


def _get_nc():
    global _NC_CACHE
    if _NC_CACHE is None:
        _NC_CACHE = build_program()
    return _NC_CACHE


def make_weights():
    import ml_dtypes
    bf = ml_dtypes.bfloat16
    dm = np.zeros((N, N), dtype=np.float32)
    for m in range(N):
        dm[m, (m + 1) % N] = 1.0
        dm[m, (m - 1) % N] = -1.0
    dyt = np.ascontiguousarray(dm.T)
    B = dyt[128:192, 128:192]
    wts = np.zeros((128, WCOLS), dtype=np.float32)
    wts[:, 0:128] = dyt[0:128, 0:128]                    # wd00
    wts[0:64, 128:256] = dyt[128:192, 0:128]             # wd10 base-0
    wts[64:128, 128:256] = dyt[128:192, 0:128]           # wd10 base-64
    wts[0:64, 256:320] = B                               # wbd blockdiag
    wts[64:128, 320:384] = B
    wts[:, 384:512] = np.eye(128)                        # wi
    wts[:, 512:640] = -np.eye(128)                       # wn
    wts[:, 640:704] = dyt[0:128, 128:192]                # wd01
    return {"wts": wts.astype(bf)}


def shard_inputs(u, T, nz=24, ncores=NCORES):
    import ml_dtypes
    bf = ml_dtypes.bfloat16
    w = make_weights()
    in_maps = []
    for k in range(ncores):
        idx = np.arange(nz * k - 2, nz * k + nz + 2) % N
        arr = np.empty((nz + 4, N, 4, N), dtype=bf)
        arr[:, :, 0:3, :] = u[:, idx, :, :].transpose(1, 2, 0, 3)
        arr[:, :, 3, :] = T[idx, :, :]
        m = {"in0": arr}
        m.update(w)
        in_maps.append(m)
    return in_maps


def kernel(u: np.ndarray, T: np.ndarray) -> np.ndarray:
    from concourse.bass_utils import run_bass_kernel_spmd

    u = np.asarray(u, dtype=np.float32)
    T = np.asarray(T, dtype=np.float32)
    nc = _get_nc()
    nz = N // NCORES
    in_maps = shard_inputs(u, T, nz=nz)
    res = run_bass_kernel_spmd(nc, in_maps, list(range(NCORES)))

    out = np.zeros((5, N, N, N), dtype=np.float32)
    for k in range(NCORES):
        o = np.asarray(res.results[k]["out"])
        out[1:5, nz * k:nz * k + nz, :, :] = o.transpose(2, 0, 1, 3) \
            .astype(np.float32)
    return out
